# revision 29
# baseline (speedup 1.0000x reference)
"""Neural ODE (RK4, 2048 steps) — TRN2 Bass kernel, 8-core data parallel.

Per core: batch 512 on the matmul free dim, activations transposed
([neuron, batch]).  Input layer runs in fp16 (t/sin/cos need range);
hidden and output layers run as fp8e4m3 DoubleRow matmuls (K=256 in one
instruction), which halves tensor-engine time vs fp16 k-tile pairs.

fp8 scaling: hidden weights are stored x16 (values ~±1, away from the
fp8 subnormal range) and the tanh activation applies scale=1/16 via its
free affine.  Output weights are stored x64; the 1/64 folds into the
RK4 combine scalars on the vector engine.

sin/cos forcing is folded into the input-layer weights per sub-eval
(host precomputed); the sin/cos/t state advances once per step via a
small fp32 rotation matmul scheduled at the end of the loop body.
"""
import numpy as np

import concourse.bacc as bacc
import concourse.bass as bass
import concourse.tile as tile
from concourse import mybir
from concourse.bass_utils import run_bass_kernel_spmd

F32 = mybir.dt.float32
FP16 = mybir.dt.float16
FP8 = mybir.dt.float8e4

DT = 0.005
NCORES = 8
BS = 512            # batch per core
NH = 256            # hidden width
NL = 3              # hidden layers
SC_H = 16.0         # hidden-weight fp8 scale

AF = mybir.ActivationFunctionType
ALU = mybir.AluOpType
DR = mybir.MatmulPerfMode.DoubleRow


def _build(steps: int) -> bass.Bass:
    nc = bacc.Bacc()

    # DRAM params (per-core)
    init_d = nc.declare_dram_parameter("init", [36, BS], F32, isOutput=False)
    wx_d = nc.declare_dram_parameter("w_x", [34, 4 * NH], F32, isOutput=False)
    wh_d = nc.declare_dram_parameter("w_h", [128, NL * 2 * NH], F32, isOutput=False)
    wo_d = nc.declare_dram_parameter("w_o", [128, 8], F32, isOutput=False)
    bh_d = nc.declare_dram_parameter("b_h", [128, 8], F32, isOutput=False)
    bo_d = nc.declare_dram_parameter("b_o", [2, 2], F32, isOutput=False)
    r2_d = nc.declare_dram_parameter("r2", [4, 3], F32, isOutput=False)
    out_d = nc.declare_dram_parameter("out", [steps * 2, BS], F32, isOutput=True)

    with tile.TileContext(nc) as tc:
        with (
            tc.tile_pool(name="cst", bufs=1) as cst,
            tc.tile_pool(name="hp", bufs=4) as hp,
            tc.tile_pool(name="tmp", bufs=4) as tmpp,
            tc.tile_pool(name="psh", bufs=4, space="PSUM") as psh,
            tc.tile_pool(name="psk", bufs=3, space="PSUM") as psk,
            tc.tile_pool(name="psr", bufs=1, space="PSUM") as psr,
        ):
            # ---- one-time loads (fp32 staging -> fp16/fp8 weight tiles) ----
            stage_wx = cst.tile([34, 4 * NH], F32)
            stage_wh = cst.tile([128, NL * 2 * NH], F32)
            stage_wo = cst.tile([128, 8], F32)
            stage_init = cst.tile([36, BS], F32)
            nc.sync.dma_start(out=stage_wx, in_=wx_d[:])
            nc.sync.dma_start(out=stage_wh, in_=wh_d[:])
            nc.sync.dma_start(out=stage_wo, in_=wo_d[:])
            nc.sync.dma_start(out=stage_init, in_=init_d[:])

            w_x = cst.tile([34, 4 * NH], FP16)
            w_h = cst.tile([128, NL * 2 * NH], FP8)
            w_o = cst.tile([128, 8], FP16)
            nc.vector.tensor_copy(w_x, stage_wx)
            nc.vector.tensor_copy(w_h, stage_wh)
            nc.vector.tensor_copy(w_o, stage_wo)

            b_h = cst.tile([128, 8], F32)
            b_o = cst.tile([2, 2], F32)
            r2 = cst.tile([4, 3], F32)
            nc.sync.dma_start(out=b_h, in_=bh_d[:])
            nc.sync.dma_start(out=b_o, in_=bo_d[:])
            nc.sync.dma_start(out=r2, in_=r2_d[:])

            # ---- persistent state ----
            # x tiles [34, BS]: rows 0-2 = sin,cos,t; row 3 = ones;
            # rows 4-31 zero (weights are zero there too); rows 32-33 = z.
            # This satisfies the base-partition-in-{0,32,..} access rule
            # for both the sin/cos/t refresh and the z updates.
            x1 = cst.tile([34, BS], FP16)
            x23 = cst.tile([34, BS], FP16)
            x4 = cst.tile([34, BS], FP16)
            u4_st = cst.tile([4, BS], F32)     # fp32 [sin, cos, t, ones]
            z_st = cst.tile([2, BS], F32)      # fp32 z state

            # dummy activation before the loop so the act-table load is
            # hoisted out of the loop body
            warm = cst.tile([1, 8], F32)
            nc.scalar.activation(out=warm, in_=stage_init[0:1, 0:8], func=AF.Tanh,
                                 bias=b_o[0:1, 0:1], scale=1.0)

            for xt in (x1, x23, x4):
                nc.vector.tensor_copy(xt, stage_init[0:34])
            nc.vector.tensor_copy(u4_st, stage_init[0:4])
            nc.vector.tensor_copy(z_st, stage_init[32:34])

            HB = BS // 2    # per-stream batch (two interleaved streams)

            def vf(j, xz_tile, kps_out, wo_off=0, k_start=True):
                """One MLP eval: xz_tile [34,BS] -> kps_out [2,BS] psum
                holding dt-scaled W_out @ h4 (bias folded elsewhere).

                The batch is split into two independent halves whose
                layers interleave: while stream 0 is in tanh, stream 1's
                matmuls keep the PE busy, and vice versa.
                """
                # input layer (fp16): one K=34 matmul per m-tile per
                # stream, both m-halves into one psum bank; the input bias
                # is folded into the ones-row weight, so one bias-free tanh
                # covers the whole bank per stream
                psin = []
                for s in range(2):
                    sl = slice(s * HB, (s + 1) * HB)
                    ps = psh.tile([128, 2 * HB], F32, tag="ps",
                                  name=f"psi{j}{s}")
                    for m in range(2):
                        nc.tensor.matmul(
                            ps[:, m * HB:(m + 1) * HB],
                            lhsT=w_x[:, j * NH + m * 128:j * NH + (m + 1) * 128],
                            rhs=xz_tile[:, sl],
                            start=True, stop=True,
                            skip_group_check=(m == 1),
                        )
                    psin.append(ps)
                hcur = []
                for s in range(2):
                    h = hp.tile([128, 2 * HB], FP8, tag="h", name=f"h{j}i{s}")
                    nc.scalar.activation(out=h, in_=psin[s], func=AF.Tanh,
                                         bias=0.0, scale=1.0)
                    hcur.append(h)
                # hidden layers: one fp8 DoubleRow matmul per m-tile per
                # stream (K=256); the last layer's tanh emits fp16 for the
                # fp16 output-layer matmuls
                for l in range(NL):
                    ps2 = [[None, None], [None, None]]
                    for m in range(2):
                        w3 = w_h[:, (l * 2 + m) * NH:(l * 2 + m + 1) * NH]
                        w3v = w3.rearrange("p (k m) -> p k m", k=2)
                        for s in range(2):
                            p = psh.tile([128, HB], F32, tag="ps",
                                         name=f"ps{j}{l}{s}{m}")
                            nc.tensor.matmul(
                                p,
                                lhsT=w3v,
                                rhs=hcur[s].rearrange("p (k n) -> p k n", k=2),
                                start=True, stop=True,
                                perf_mode=DR,
                            )
                            ps2[s][m] = p
                    hdt = FP16 if l == NL - 1 else FP8
                    hnew = []
                    for s in range(2):
                        h2 = hp.tile([128, 2 * HB], hdt, tag="h",
                                     name=f"h{j}{l}{s}")
                        for m in range(2):
                            nc.scalar.activation(
                                out=h2[:, m * HB:(m + 1) * HB],
                                in_=ps2[s][m], func=AF.Tanh,
                                bias=b_h[:, 2 * l + m:2 * l + m + 1],
                                scale=float(1.0 / SC_H),
                            )
                        hnew.append(h2)
                    hcur = hnew
                # output layer (fp16, dt-scales folded into w_o).
                # start=True clears has_written for the WHOLE bank, so only
                # the very first matmul touching this bank per iteration may
                # set it; stream 1 relies on region-level has_written bits
                # (clear -> overwrite, set -> accumulate).
                for s in range(2):
                    sl = slice(s * HB, (s + 1) * HB)
                    for kt in range(2):
                        first = (s == 0 and kt == 0 and k_start)
                        nc.tensor.matmul(
                            kps_out[:, sl],
                            lhsT=w_o[:, wo_off + kt * 2:wo_off + (kt + 1) * 2],
                            rhs=hcur[s][:, kt * HB:(kt + 1) * HB],
                            start=first, stop=(s == 1 and kt == 1),
                            skip_group_check=not first,
                        )

            with tc.For_i(0, steps * 2, 2, staggered_reset=True) as iv:
                # k1 (psum = (dt/2)*W_out@h4 — b_out folded into next L_in)
                k1p = psk.tile([2, BS], F32, tag="kps")
                vf(0, xz1, k1p, wo_off=0)
                nc.vector.tensor_add(xz23[0:2], z_st, k1p)   # za = z + (dt/2)k1
                # k2
                k2p = psk.tile([2, BS], F32, tag="kps")
                vf(1, xz23, k2p, wo_off=0)
                nc.vector.tensor_add(xz23[0:2], z_st, k2p)   # zb = z + (dt/2)k2
                # k3 (psum = dt*W_out@h4)
                k34p = psk.tile([2, BS], F32, tag="kps")
                vf(2, xz23, k34p, wo_off=4)
                nc.vector.tensor_add(xz4[0:2], z_st, k34p)   # zc = z + dt*k3
                # k4 accumulates into k34p: p34 = dt*k3 + (dt/2)*k4
                vf(3, xz4, k34p, wo_off=0, k_start=False)

                nc.vector.tensor_copy(x23[0:3], rot_ps)

                # z' = z + (1/3)p1 + (2/3)p2 + (1/3)p34 + dt*b_o
                u1 = tmpp.tile([2, BS], F32, tag="tmp")
                nc.vector.tensor_scalar(
                    out=u1, in0=k1p, scalar1=b_o[:, 1:2], scalar2=float(1.0 / 3.0),
                    op0=ALU.add, op1=ALU.mult,
                )
                u2 = tmpp.tile([2, BS], F32, tag="tmp")
                nc.vector.tensor_add(u2, z_st, u1)
                u3 = tmpp.tile([2, BS], F32, tag="tmp")
                nc.vector.tensor_scalar_mul(u3, k2p, float(2.0 / 3.0))
                u4 = tmpp.tile([2, BS], F32, tag="tmp")
                nc.vector.tensor_add(u4, u2, u3)
                u5 = tmpp.tile([2, BS], F32, tag="tmp")
                nc.vector.tensor_scalar_mul(u5, k34p, float(1.0 / 3.0))
                nc.vector.tensor_add(z_st, u4, u5)

                # state updates for next step
                nc.vector.tensor_copy(xz1[0:2], z_st)
                nc.vector.tensor_copy(u4_st[0:3], rot_ps)
                nc.vector.tensor_copy(stz, rot_ps)

                # store z' trajectory
                nc.sync.dma_start(out=out_d[bass.ds(iv, 2)], in_=z_st)

    nc.compile()
    return nc


def _prep_inputs(z0, t0, W_in, b_in, W_h, b_h, W_out, b_out):
    f64 = np.float64
    W_in = W_in.astype(f64)
    cs = [0.0, DT / 2.0, DT / 2.0, DT]

    # w_stc: [3, 4*NH]: variant j, rows (sin, cos, t)
    # w_zb:  [3, 4*NH]: variant j, rows (z0, z1, bias)
    w_stc = np.zeros((3, 4 * NH), f64)
    w_zb = np.zeros((3, 4 * NH), f64)
    zfold = W_in[:, 1:3] @ b_out.astype(f64)    # per unit b_out scale
    zc_scale = [0.0, DT / 2.0, DT / 2.0, DT]
    for j, c in enumerate(cs):
        col_sin = W_in[:, 3] * np.cos(c) - W_in[:, 4] * np.sin(c)
        col_cos = W_in[:, 3] * np.sin(c) + W_in[:, 4] * np.cos(c)
        sl = slice(j * NH, (j + 1) * NH)
        w_stc[0, sl] = col_sin
        w_stc[1, sl] = col_cos
        w_stc[2, sl] = W_in[:, 0]
        w_zb[0, sl] = W_in[:, 1]
        w_zb[1, sl] = W_in[:, 2]
        w_zb[2, sl] = b_in.astype(f64) + c * W_in[:, 0] + zc_scale[j] * zfold

    # w_h packed for DoubleRow: per (l, m): [kp, (kt, mf)], scaled by SC_H
    wh = np.zeros((128, NL * 2 * NH), f64)
    for l in range(NL):
        for m in range(2):
            blk = np.zeros((128, 2, 128), f64)
            for kt in range(2):
                # blk[kp, kt, mf] = W_h[l][m*128+mf, kt*128+kp]
                blk[:, kt, :] = W_h[l][m * 128:(m + 1) * 128,
                                       kt * 128:(kt + 1) * 128].T.astype(f64)
            wh[:, (l * 2 + m) * NH:(l * 2 + m + 1) * NH] = \
                blk.reshape(128, 2 * 128) * SC_H

    # w_o: [kp, (kt, c)] col pairs, fp16, dt-scales folded
    wo_base = W_out.T.reshape(2, 128, 2).transpose(1, 0, 2).reshape(128, 4).astype(f64)
    wo = np.concatenate([wo_base * (DT / 2.0), wo_base * DT], 1)  # [128, 8]

    # hidden-layer biases: [128, (l, m)]
    bh = np.zeros((128, 8), np.float64)
    for l in range(NL):
        bh[:, 2 * l] = b_h[l][:128]
        bh[:, 2 * l + 1] = b_h[l][128:]

    bo = np.stack([b_out.astype(f64), 3.0 * DT * b_out.astype(f64)], 1)  # [2,2]

    # lhsT [k=(sin,cos,t,one), m=(sin',cos',t')]
    r2 = np.array([
        [np.cos(DT), -np.sin(DT), 0.0],
        [np.sin(DT), np.cos(DT), 0.0],
        [0.0, 0.0, 1.0],
        [0.0, 0.0, DT],
    ], f64)

    common = {
        "w_stc": w_stc.astype(np.float32),
        "w_zb": w_zb.astype(np.float32),
        "w_h": wh.astype(np.float32),
        "w_o": wo.astype(np.float32),
        "b_h": bh.astype(np.float32),
        "b_o": bo.astype(np.float32),
        "r2": r2.astype(np.float32),
    }

    in_maps = []
    for c in range(NCORES):
        sl = slice(c * BS, (c + 1) * BS)
        t0c = t0[sl, 0].astype(np.float32)
        z0c = z0[sl].astype(np.float32)
        init = np.zeros((36, BS), np.float32)
        init[0] = np.sin(t0c)
        init[1] = np.cos(t0c)
        init[2] = t0c
        init[3] = 1.0
        init[32] = z0c[:, 0]
        init[33] = z0c[:, 1]
        init[34] = 1.0
        in_maps.append({**common, "init": init})
    return in_maps


_CACHE = {}


def _get_nc(steps):
    if steps not in _CACHE:
        _CACHE[steps] = _build(steps)
    return _CACHE[steps]


def kernel(z0, t0, W_in, b_in, W_h, b_h, W_out, b_out, steps, trace=False):
    steps = int(steps)
    nc = _get_nc(steps)
    in_maps = _prep_inputs(
        np.asarray(z0), np.asarray(t0), np.asarray(W_in), np.asarray(b_in),
        np.asarray(W_h), np.asarray(b_h), np.asarray(W_out), np.asarray(b_out),
    )
    res = run_bass_kernel_spmd(nc, in_maps, list(range(NCORES)), trace=trace)
    outs = []
    for c in range(NCORES):
        o = res.results[c]["out"].reshape(steps, 2, BS)
        outs.append(np.ascontiguousarray(o.transpose(2, 0, 1)))
    full = np.concatenate(outs, 0).astype(np.float32)
    if trace:
        kernel.last_results = res
    return full


# revision 30
# speedup vs baseline: 1.2012x; 1.2012x over previous
"""Neural ODE (RK4, 2048 steps) — TRN2 Bass kernel, 8-core data parallel.

Per core: batch 512 on the matmul free dim, activations transposed
([neuron, batch]).  Input layer runs in fp16 (t/sin/cos need range);
hidden and output layers run as fp8e4m3 DoubleRow matmuls (K=256 in one
instruction), which halves tensor-engine time vs fp16 k-tile pairs.

fp8 scaling: hidden weights are stored x16 (values ~±1, away from the
fp8 subnormal range) and the tanh activation applies scale=1/16 via its
free affine.  Output weights are stored x64; the 1/64 folds into the
RK4 combine scalars on the vector engine.

sin/cos forcing is folded into the input-layer weights per sub-eval
(host precomputed); the sin/cos/t state advances once per step via a
small fp32 rotation matmul scheduled at the end of the loop body.
"""
import numpy as np

import concourse.bacc as bacc
import concourse.bass as bass
import concourse.tile as tile
from concourse import mybir
from concourse.bass_utils import run_bass_kernel_spmd

F32 = mybir.dt.float32
FP16 = mybir.dt.float16
FP8 = mybir.dt.float8e4

DT = 0.005
NCORES = 8
BS = 512            # batch per core
NH = 256            # hidden width
NL = 3              # hidden layers
SC_H = 16.0         # hidden-weight fp8 scale

AF = mybir.ActivationFunctionType
ALU = mybir.AluOpType
DR = mybir.MatmulPerfMode.DoubleRow


def _build(steps: int) -> bass.Bass:
    nc = bacc.Bacc()

    # DRAM params (per-core)
    init_d = nc.declare_dram_parameter("init", [36, BS], F32, isOutput=False)
    wx_d = nc.declare_dram_parameter("w_x", [34, 4 * NH], F32, isOutput=False)
    wh_d = nc.declare_dram_parameter("w_h", [128, NL * 2 * NH], F32, isOutput=False)
    wo_d = nc.declare_dram_parameter("w_o", [128, 8], F32, isOutput=False)
    bh_d = nc.declare_dram_parameter("b_h", [128, 8], F32, isOutput=False)
    bo_d = nc.declare_dram_parameter("b_o", [2, 2], F32, isOutput=False)
    r2_d = nc.declare_dram_parameter("r2", [4, 3], F32, isOutput=False)
    out_d = nc.declare_dram_parameter("out", [steps * 2, BS], F32, isOutput=True)

    with tile.TileContext(nc) as tc:
        with (
            tc.tile_pool(name="cst", bufs=1) as cst,
            tc.tile_pool(name="hp", bufs=4) as hp,
            tc.tile_pool(name="tmp", bufs=4) as tmpp,
            tc.tile_pool(name="psh", bufs=4, space="PSUM") as psh,
            tc.tile_pool(name="psk", bufs=3, space="PSUM") as psk,
            tc.tile_pool(name="psr", bufs=1, space="PSUM") as psr,
        ):
            # ---- one-time loads (fp32 staging -> fp16/fp8 weight tiles) ----
            stage_wx = cst.tile([34, 4 * NH], F32)
            stage_wh = cst.tile([128, NL * 2 * NH], F32)
            stage_wo = cst.tile([128, 8], F32)
            stage_init = cst.tile([36, BS], F32)
            nc.sync.dma_start(out=stage_wx, in_=wx_d[:])
            nc.sync.dma_start(out=stage_wh, in_=wh_d[:])
            nc.sync.dma_start(out=stage_wo, in_=wo_d[:])
            nc.sync.dma_start(out=stage_init, in_=init_d[:])

            w_x = cst.tile([34, 4 * NH], FP16)
            w_h = cst.tile([128, NL * 2 * NH], FP8)
            w_o = cst.tile([128, 8], FP16)
            nc.vector.tensor_copy(w_x, stage_wx)
            nc.vector.tensor_copy(w_h, stage_wh)
            nc.vector.tensor_copy(w_o, stage_wo)

            b_h = cst.tile([128, 8], F32)
            b_o = cst.tile([2, 2], F32)
            r2 = cst.tile([4, 3], F32)
            nc.sync.dma_start(out=b_h, in_=bh_d[:])
            nc.sync.dma_start(out=b_o, in_=bo_d[:])
            nc.sync.dma_start(out=r2, in_=r2_d[:])

            # ---- persistent state ----
            # x tiles [34, BS]: rows 0-2 = sin,cos,t; row 3 = ones;
            # rows 4-31 zero (weights are zero there too); rows 32-33 = z.
            # This satisfies the base-partition-in-{0,32,..} access rule
            # for both the sin/cos/t refresh and the z updates.
            x1 = cst.tile([34, BS], FP16)
            x23 = cst.tile([34, BS], FP16)
            x4 = cst.tile([34, BS], FP16)
            u4_st = cst.tile([4, BS], F32)     # fp32 [sin, cos, t, ones]
            z_st = cst.tile([2, BS], F32)      # fp32 z state

            # dummy activation before the loop so the act-table load is
            # hoisted out of the loop body
            warm = cst.tile([1, 8], F32)
            nc.scalar.activation(out=warm, in_=stage_init[0:1, 0:8], func=AF.Tanh,
                                 bias=b_o[0:1, 0:1], scale=1.0)

            for xt in (x1, x23, x4):
                nc.vector.tensor_copy(xt, stage_init[0:34])
            nc.vector.tensor_copy(u4_st, stage_init[0:4])
            nc.vector.tensor_copy(z_st, stage_init[32:34])

            HB = BS // 2    # per-stream batch (two interleaved streams)

            def vf(j, xz_tile, kps_out, wo_off=0, k_start=True):
                """One MLP eval: xz_tile [34,BS] -> kps_out [2,BS] psum
                holding dt-scaled W_out @ h4 (bias folded elsewhere).

                The batch is split into two independent halves whose
                layers interleave: while stream 0 is in tanh, stream 1's
                matmuls keep the PE busy, and vice versa.
                """
                # input layer (fp16): one K=34 matmul per m-tile per
                # stream, both m-halves into one psum bank; the input bias
                # is folded into the ones-row weight, so one bias-free tanh
                # covers the whole bank per stream
                psin = []
                for s in range(2):
                    sl = slice(s * HB, (s + 1) * HB)
                    ps = psh.tile([128, 2 * HB], F32, tag="ps",
                                  name=f"psi{j}{s}")
                    for m in range(2):
                        nc.tensor.matmul(
                            ps[:, m * HB:(m + 1) * HB],
                            lhsT=w_x[:, j * NH + m * 128:j * NH + (m + 1) * 128],
                            rhs=xz_tile[:, sl],
                            start=True, stop=True,
                            skip_group_check=(m == 1),
                        )
                    psin.append(ps)
                hcur = []
                for s in range(2):
                    h = hp.tile([128, 2 * HB], FP8, tag="h", name=f"h{j}i{s}")
                    nc.scalar.activation(out=h, in_=psin[s], func=AF.Tanh,
                                         bias=0.0, scale=1.0)
                    hcur.append(h)
                # hidden layers: one fp8 DoubleRow matmul per m-tile per
                # stream (K=256); the last layer's tanh emits fp16 for the
                # fp16 output-layer matmuls
                for l in range(NL):
                    ps2 = [[None, None], [None, None]]
                    for m in range(2):
                        w3 = w_h[:, (l * 2 + m) * NH:(l * 2 + m + 1) * NH]
                        w3v = w3.rearrange("p (k m) -> p k m", k=2)
                        for s in range(2):
                            p = psh.tile([128, HB], F32, tag="ps",
                                         name=f"ps{j}{l}{s}{m}")
                            nc.tensor.matmul(
                                p,
                                lhsT=w3v,
                                rhs=hcur[s].rearrange("p (k n) -> p k n", k=2),
                                start=True, stop=True,
                                perf_mode=DR,
                            )
                            ps2[s][m] = p
                    hdt = FP16 if l == NL - 1 else FP8
                    hnew = []
                    for s in range(2):
                        h2 = hp.tile([128, 2 * HB], hdt, tag="h",
                                     name=f"h{j}{l}{s}")
                        for m in range(2):
                            nc.scalar.activation(
                                out=h2[:, m * HB:(m + 1) * HB],
                                in_=ps2[s][m], func=AF.Tanh,
                                bias=b_h[:, 2 * l + m:2 * l + m + 1],
                                scale=float(1.0 / SC_H),
                            )
                        hnew.append(h2)
                    hcur = hnew
                # output layer (fp16, dt-scales folded into w_o).
                # start=True clears has_written for the WHOLE bank, so only
                # the very first matmul touching this bank per iteration may
                # set it; stream 1 relies on region-level has_written bits
                # (clear -> overwrite, set -> accumulate).
                for s in range(2):
                    sl = slice(s * HB, (s + 1) * HB)
                    for kt in range(2):
                        first = (s == 0 and kt == 0 and k_start)
                        nc.tensor.matmul(
                            kps_out[:, sl],
                            lhsT=w_o[:, wo_off + kt * 2:wo_off + (kt + 1) * 2],
                            rhs=hcur[s][:, kt * HB:(kt + 1) * HB],
                            start=first, stop=(s == 1 and kt == 1),
                            skip_group_check=not first,
                        )

            with tc.For_i(0, steps * 2, 2, staggered_reset=True) as iv:
                # k1 (psum = (dt/2)*W_out@h4 — b_out folded into next L_in)
                k1p = psk.tile([2, BS], F32, tag="kps")
                vf(0, xz1, k1p, wo_off=0)
                nc.vector.tensor_add(xz23[0:2], z_st, k1p)   # za = z + (dt/2)k1
                # k2
                k2p = psk.tile([2, BS], F32, tag="kps")
                vf(1, xz23, k2p, wo_off=0)
                nc.vector.tensor_add(xz23[0:2], z_st, k2p)   # zb = z + (dt/2)k2
                # k3 (psum = dt*W_out@h4)
                k34p = psk.tile([2, BS], F32, tag="kps")
                vf(2, xz23, k34p, wo_off=4)
                nc.vector.tensor_add(xz4[0:2], z_st, k34p)   # zc = z + dt*k3
                # k4 accumulates into k34p: p34 = dt*k3 + (dt/2)*k4
                vf(3, xz4, k34p, wo_off=0, k_start=False)

                # [sin,cos,t] advance by dt (fp32 matmul), off the critical
                # path: emitted after the k-chain
                rot_ps = psr.tile([3, BS], F32, tag="rot")
                nc.tensor.matmul(rot_ps, lhsT=r2, rhs=u4_st, start=True, stop=True)

                # z' = z + (1/3)p1 + (2/3)p2 + (1/3)p34 + dt*b_o
                u1 = tmpp.tile([2, BS], F32, tag="tmp")
                nc.vector.tensor_scalar(
                    out=u1, in0=k1p, scalar1=b_o[:, 1:2], scalar2=float(1.0 / 3.0),
                    op0=ALU.add, op1=ALU.mult,
                )
                u2 = tmpp.tile([2, BS], F32, tag="tmp")
                nc.vector.tensor_add(u2, z_st, u1)
                u3 = tmpp.tile([2, BS], F32, tag="tmp")
                nc.vector.tensor_scalar_mul(u3, k2p, float(2.0 / 3.0))
                u4 = tmpp.tile([2, BS], F32, tag="tmp")
                nc.vector.tensor_add(u4, u2, u3)
                u5 = tmpp.tile([2, BS], F32, tag="tmp")
                nc.vector.tensor_scalar_mul(u5, k34p, float(1.0 / 3.0))
                nc.vector.tensor_add(z_st, u4, u5)

                # state updates for next step
                nc.vector.tensor_copy(xz1[0:2], z_st)
                nc.vector.tensor_copy(u4_st[0:3], rot_ps)
                nc.vector.tensor_copy(stz, rot_ps)

                # store z' trajectory
                nc.sync.dma_start(out=out_d[bass.ds(iv, 2)], in_=z_st)

    nc.compile()
    return nc


def _prep_inputs(z0, t0, W_in, b_in, W_h, b_h, W_out, b_out):
    f64 = np.float64
    W_in = W_in.astype(f64)
    cs = [0.0, DT / 2.0, DT / 2.0, DT]

    # w_stc: [3, 4*NH]: variant j, rows (sin, cos, t)
    # w_zb:  [3, 4*NH]: variant j, rows (z0, z1, bias)
    w_stc = np.zeros((3, 4 * NH), f64)
    w_zb = np.zeros((3, 4 * NH), f64)
    zfold = W_in[:, 1:3] @ b_out.astype(f64)    # per unit b_out scale
    zc_scale = [0.0, DT / 2.0, DT / 2.0, DT]
    for j, c in enumerate(cs):
        col_sin = W_in[:, 3] * np.cos(c) - W_in[:, 4] * np.sin(c)
        col_cos = W_in[:, 3] * np.sin(c) + W_in[:, 4] * np.cos(c)
        sl = slice(j * NH, (j + 1) * NH)
        w_stc[0, sl] = col_sin
        w_stc[1, sl] = col_cos
        w_stc[2, sl] = W_in[:, 0]
        w_zb[0, sl] = W_in[:, 1]
        w_zb[1, sl] = W_in[:, 2]
        w_zb[2, sl] = b_in.astype(f64) + c * W_in[:, 0] + zc_scale[j] * zfold

    # w_h packed for DoubleRow: per (l, m): [kp, (kt, mf)], scaled by SC_H
    wh = np.zeros((128, NL * 2 * NH), f64)
    for l in range(NL):
        for m in range(2):
            blk = np.zeros((128, 2, 128), f64)
            for kt in range(2):
                # blk[kp, kt, mf] = W_h[l][m*128+mf, kt*128+kp]
                blk[:, kt, :] = W_h[l][m * 128:(m + 1) * 128,
                                       kt * 128:(kt + 1) * 128].T.astype(f64)
            wh[:, (l * 2 + m) * NH:(l * 2 + m + 1) * NH] = \
                blk.reshape(128, 2 * 128) * SC_H

    # w_o: [kp, (kt, c)] col pairs, fp16, dt-scales folded
    wo_base = W_out.T.reshape(2, 128, 2).transpose(1, 0, 2).reshape(128, 4).astype(f64)
    wo = np.concatenate([wo_base * (DT / 2.0), wo_base * DT], 1)  # [128, 8]

    # hidden-layer biases: [128, (l, m)]
    bh = np.zeros((128, 8), np.float64)
    for l in range(NL):
        bh[:, 2 * l] = b_h[l][:128]
        bh[:, 2 * l + 1] = b_h[l][128:]

    bo = np.stack([b_out.astype(f64), 3.0 * DT * b_out.astype(f64)], 1)  # [2,2]

    # lhsT [k=(sin,cos,t,one), m=(sin',cos',t')]
    r2 = np.array([
        [np.cos(DT), -np.sin(DT), 0.0],
        [np.sin(DT), np.cos(DT), 0.0],
        [0.0, 0.0, 1.0],
        [0.0, 0.0, DT],
    ], f64)

    common = {
        "w_stc": w_stc.astype(np.float32),
        "w_zb": w_zb.astype(np.float32),
        "w_h": wh.astype(np.float32),
        "w_o": wo.astype(np.float32),
        "b_h": bh.astype(np.float32),
        "b_o": bo.astype(np.float32),
        "r2": r2.astype(np.float32),
    }

    in_maps = []
    for c in range(NCORES):
        sl = slice(c * BS, (c + 1) * BS)
        t0c = t0[sl, 0].astype(np.float32)
        z0c = z0[sl].astype(np.float32)
        init = np.zeros((36, BS), np.float32)
        init[0] = np.sin(t0c)
        init[1] = np.cos(t0c)
        init[2] = t0c
        init[3] = 1.0
        init[32] = z0c[:, 0]
        init[33] = z0c[:, 1]
        init[34] = 1.0
        in_maps.append({**common, "init": init})
    return in_maps


_CACHE = {}


def _get_nc(steps):
    if steps not in _CACHE:
        _CACHE[steps] = _build(steps)
    return _CACHE[steps]


def kernel(z0, t0, W_in, b_in, W_h, b_h, W_out, b_out, steps, trace=False):
    steps = int(steps)
    nc = _get_nc(steps)
    in_maps = _prep_inputs(
        np.asarray(z0), np.asarray(t0), np.asarray(W_in), np.asarray(b_in),
        np.asarray(W_h), np.asarray(b_h), np.asarray(W_out), np.asarray(b_out),
    )
    res = run_bass_kernel_spmd(nc, in_maps, list(range(NCORES)), trace=trace)
    outs = []
    for c in range(NCORES):
        o = res.results[c]["out"].reshape(steps, 2, BS)
        outs.append(np.ascontiguousarray(o.transpose(2, 0, 1)))
    full = np.concatenate(outs, 0).astype(np.float32)
    if trace:
        kernel.last_results = res
    return full
